# revision 1
# baseline (speedup 1.0000x reference)
"""Multi-head self-attention Trainium2 kernel (8 NeuronCores).

Problem: B=4, S=2048, D=1024, H=8 heads (HD=128).
  qkv = x @ qkv_w.T + qkv_b ; q,k,v = split(qkv)
  q = (q @ q_w.T + q_b)  (same k, v) -> [B,H,S,HD]
  scores = q k^T * HD^-0.5, masked softmax (attn_mask==1 -> -inf), o = attn @ v
  out = o @ out_w.T + out_b

Sharding: 8 cores = 4 batches x 2 head-groups (4 heads each).
Core c: batch b = c % 4, head-group g = c // 4.

Host-side algebraic folding: the qkv projection and per-stream q/k/v
projections are both linear, so they are composed into single effective
weights (W_eff = w @ qkv_w_slice), halving device matmul work. The
out-projection is row-parallel across head-groups; the two partial
outputs per batch are summed on host (the unshard step) with out_b.

Device flow per core (all matmuls bf16 with fp32 PSUM accumulation):
  qT_h[HD,S], kT_h[HD,S] = W x^T      (contraction over D on partitions)
  v[S, 4*HD]                          (natural layout)
  per head, per q-half (1024 q), software-pipelined 2 chunks deep:
    for kc in 16 k-chunks:
      sT = kT_h[:,kc]^T @ qT_h        [128 k, 1024 q]   (PE -> PSUM f32)
      p  = exp(SCALE * sT)            (ACT -> bf16 SBUF)
      pm = p * keepT[kc]              (DVE; keep = attn_mask.T == 0)
      oT += v[kc]^T-as-lhsT @ pm      -> oT[HD, q]      (PE, PSUM accum)
      dB += ones^T @ pm               broadcast denominator (PE, PSUM)
    oT_sb = oT * exp(-ln(dB))         softmax normalization (ACT+DVE -> bf16)
  out_partial[s,:] = sum_h oT_h[:,s_chunk]^T @ outwT_h   (+host bias/sum)
"""

import os
import sys
import types

sys.path.insert(0, "/opt/trn_rl_repo")

import numpy as np
import ml_dtypes

BF16 = ml_dtypes.bfloat16

B, S, D, H, HD = 4, 2048, 1024, 8, 128
HG = 2           # head groups
HPG = H // HG    # heads per group (4)
GD = HPG * HD    # dims per group (512)
SCALE = float(HD) ** -0.5
NKC = S // 128   # 16 k chunks
NSC = S // 128   # 16 s chunks
ND = D // 128    # 8 d chunks

_cached = {}


def _install_ntff_hook_shim():
    """The agent image's antenv lacks axon_hooks; shim it so trace works."""
    if "antenv.axon_hooks" in sys.modules:
        return
    try:
        import trn_agent_boot.trn_boot as _tb

        _hook = _tb._ntff_profile_via_ctypes("/opt/axon/libaxon_pjrt.so")
    except Exception:
        _hook = None
    _m = types.ModuleType("antenv.axon_hooks")
    _m.get_axon_ntff_profile_hook = lambda: _hook
    sys.modules["antenv.axon_hooks"] = _m


def _split_waits(nc, mybir, maxw=1):
    """Walrus in this image allows only one sync wait per instruction;
    hoist extra waits onto preceding NoOps on the same engine."""
    n_new = 0
    for fn in nc.m.functions:
        for bb in fn.blocks:
            newlist = []
            for inst in bb.instructions:
                si = inst.sync_info
                if si is not None and si.on_wait is not None and len(si.on_wait) > maxw:
                    waits = list(si.on_wait)
                    extra, keep = waits[:-maxw], waits[-maxw:]
                    while extra:
                        chunk, extra = extra[:maxw], extra[maxw:]
                        nop = mybir.InstNoOp(name=f"I-waitsplit-{nc.next_id()}")
                        nop.engine = inst.engine
                        nop.sync_info = mybir.SyncInfo(on_wait=chunk, on_update=[])
                        newlist.append(nop)
                        n_new += 1
                    si.on_wait = keep
                newlist.append(inst)
            bb.instructions = newlist
    return n_new


def _build_program(use_vbias=True):
    import concourse.bass as bass
    import concourse.mybir as mybir
    import concourse.tile as tile

    f32 = mybir.dt.float32
    bf16 = mybir.dt.bfloat16
    Exp = mybir.ActivationFunctionType.Exp
    Ident = mybir.ActivationFunctionType.Identity
    Ln = mybir.ActivationFunctionType.Ln

    nc = bass.Bass()

    # DRAM parameters (per-core shards, pre-tiled on host)
    xT = nc.declare_dram_parameter("xT", [ND, 128, S], bf16, isOutput=False)
    wqT = nc.declare_dram_parameter("wqT", [ND, 128, GD], bf16, isOutput=False)
    wkT = nc.declare_dram_parameter("wkT", [ND, 128, GD], bf16, isOutput=False)
    wvT = nc.declare_dram_parameter("wvT", [ND, 128, GD], bf16, isOutput=False)
    bq = nc.declare_dram_parameter("bq", [128, HPG], f32, isOutput=False)
    bk = nc.declare_dram_parameter("bk", [128, HPG], f32, isOutput=False)
    bvrow = nc.declare_dram_parameter("bvrow", [1, GD], bf16, isOutput=False)
    outwT = nc.declare_dram_parameter("outwT", [HPG, 128, D], bf16, isOutput=False)
    keepT = nc.declare_dram_parameter("keepT", [NKC, 128, S], bf16, isOutput=False)
    out = nc.declare_dram_parameter("out", [S, D], f32, isOutput=True)

    with tile.TileContext(nc) as tc:
        import contextlib

        with contextlib.ExitStack() as ctx:
            # --- pools ---
            # xT and keepT share one 16-slot rotation of [128, S] bf16 tiles.
            p_big = ctx.enter_context(tc.tile_pool(name="big2k", bufs=16))
            p_pers = ctx.enter_context(tc.tile_pool(name="pers", bufs=1))
            p_pm = ctx.enter_context(tc.tile_pool(name="pm", bufs=10))
            p_sm = ctx.enter_context(tc.tile_pool(name="small", bufs=2))
            pp_big = ctx.enter_context(tc.tile_pool(name="ppbig", bufs=2, space="PSUM"))
            pp_sm = ctx.enter_context(tc.tile_pool(name="ppsm", bufs=4, space="PSUM"))

            # --- constants + small inputs ---
            ones128 = p_pers.tile([128, 128], bf16, tag="ones128", name="ones128")
            nc.vector.memset(ones128, 1.0)

            # --- batched loads: few large DMAs (each dma_start costs ~600ns
            # of sequencer issue time); q weights first, then xT in 4 chunks
            # so the q projection groups start after ~1 chunk ---
            w_sb = {}
            xt_tiles = []
            for d in range(ND):
                t = p_pers.tile([128, GD], bf16, tag=f"wq{d}", name=f"wq{d}")
                nc.sync.dma_start(out=t, in_=wqT[d])
                w_sb[("q", d)] = t
                t = p_big.tile([128, S], bf16, tag="big2k", name="big2k")
                nc.sync.dma_start(out=t, in_=xT[d])
                xt_tiles.append(t)
            for name, drm in (("k", wkT), ("v", wvT)):
                for d in range(ND):
                    t = p_pers.tile([128, GD], bf16, tag=f"w{name}{d}", name=f"w{name}{d}")
                    nc.sync.dma_start(out=t, in_=drm[d])
                    w_sb[(name, d)] = t

            bq_sb = p_pers.tile([128, HPG], f32, tag="bq", name="bq_sb")
            nc.sync.dma_start(out=bq_sb, in_=bq[:, :])
            bk_sb = p_pers.tile([128, HPG], f32, tag="bk", name="bk_sb")
            nc.sync.dma_start(out=bk_sb, in_=bk[:, :])
            bv_sb = None
            if use_vbias:
                bv_sb = p_pers.tile([1, GD], bf16, tag="bv", name="bv_sb")
                nc.sync.dma_start(out=bv_sb, in_=bvrow[:, :])

            def w_sl(name, d):
                return w_sb[(name, d)]

            def xT_sl(d, lo, hi):
                return xt_tiles[d][:, lo:hi]

            keep_tiles = [None] * NKC
            for kc in range(8):
                t = p_big.tile([128, S], bf16, tag="big2k", name="big2k")
                nc.sync.dma_start(out=t, in_=keepT[kc])
                keep_tiles[kc] = t

            def keep_sl(kc, lo, hi):
                return keep_tiles[kc][:, lo:hi]

            outw_sb = []
            for h in range(HPG):
                t = p_pers.tile([128, D], bf16, tag=f"outw{h}", name=f"outw{h}")
                nc.sync.dma_start(out=t, in_=outwT[h])
                outw_sb.append(t)

            # --- projections (d-major, 4 concurrent PSUM accumulators so the
            # d=0 matmuls of a group start as soon as xT[0]/w[0] land) ---
            qT_sb = [p_pers.tile([128, S], bf16, tag=f"qT{h}", name=f"qT{h}") for h in range(HPG)]
            kT_sb = [p_pers.tile([128, S], bf16, tag=f"kT{h}", name=f"kT{h}") for h in range(HPG)]

            units = []  # (stream, head, quarter)
            for name, dst, bias in (("q", qT_sb, bq_sb), ("k", kT_sb, bk_sb)):
                for h in range(HPG):
                    for qu in range(4):
                        units.append((name, dst, bias, h, qu))
            for gstart in range(0, len(units), 4):
                group = units[gstart:gstart + 4]
                pss = [
                    pp_sm.tile([128, 512], f32, tag="ppsm", name="ppsm")
                    for _ in group
                ]
                for d in range(ND):
                    for (name, dst, bias, h, qu), ps in zip(group, pss):
                        nc.tensor.matmul(
                            ps,
                            lhsT=w_sl(name, d)[:, h * 128:(h + 1) * 128],
                            rhs=xT_sl(d, qu * 512, (qu + 1) * 512),
                            start=(d == 0),
                            stop=(d == ND - 1),
                        )
                for (name, dst, bias, h, qu), ps in zip(group, pss):
                    nc.scalar.activation(
                        out=dst[h][:, qu * 512:(qu + 1) * 512],
                        in_=ps,
                        func=Ident,
                        bias=bias[:, h:h + 1],
                    )

            v_sb = [p_pers.tile([128, GD], bf16, tag=f"v{sc}", name=f"v{sc}") for sc in range(NSC)]
            for sc in range(NSC):
                ps = pp_sm.tile([128, GD], f32, tag="ppsm", name="ppsm")
                for d in range(ND):
                    nc.tensor.matmul(
                        ps,
                        lhsT=xT_sl(d, sc * 128, (sc + 1) * 128),
                        rhs=w_sl("v", d),
                        start=(d == 0),
                        stop=(d == ND - 1) and not use_vbias,
                    )
                if use_vbias:
                    # bias via K=1 ones row
                    nc.tensor.matmul(
                        ps,
                        lhsT=ones128[0:1, :],
                        rhs=bv_sb,
                        start=False,
                        stop=True,
                    )
                nc.vector.tensor_copy(v_sb[sc], ps)

            # --- second half of keepT (reuses xT slots once proj done) ---
            for kc in range(8, NKC):
                t = p_big.tile([128, S], bf16, tag="big2k", name="big2k")
                nc.sync.dma_start(out=t, in_=keepT[kc])
                keep_tiles[kc] = t

            # --- attention ---
            oT_sb = [p_pers.tile([128, S], bf16, tag=f"oT{h}", name=f"oT{h}") for h in range(HPG)]
            for h in range(HPG):
                for half in range(2):
                    q0 = half * 1024
                    o_ps = [pp_sm.tile([128, 512], f32, tag="ppsm", name="ppsm") for _ in range(2)]
                    d_ps = [pp_sm.tile([128, 512], f32, tag="ppsm", name="ppsm") for _ in range(2)]

                    def consume(kc, pm):
                        # oT/dB accumulation for chunk kc, issued two stages
                        # late so the PE never waits on ACT/DVE for this kc
                        for qq in range(2):
                            nc.tensor.matmul(
                                o_ps[qq],
                                lhsT=v_sb[kc][:, h * 128:(h + 1) * 128],
                                rhs=pm[:, qq * 512:(qq + 1) * 512],
                                start=(kc == 0),
                                stop=(kc == NKC - 1),
                            )
                        for qq in range(2):
                            nc.tensor.matmul(
                                d_ps[qq],
                                lhsT=ones128,
                                rhs=pm[:, qq * 512:(qq + 1) * 512],
                                start=(kc == 0),
                                stop=(kc == NKC - 1),
                            )

                    pending = []  # [(kc, pm)] — 2-stage delay
                    for kc in range(NKC):
                        sT = pp_big.tile([128, 1024], f32, tag="ppbig", name="ppbig")
                        for nn in range(2):
                            nc.tensor.matmul(
                                sT[:, nn * 512:(nn + 1) * 512],
                                lhsT=kT_sb[h][:, kc * 128:(kc + 1) * 128],
                                rhs=qT_sb[h][:, q0 + nn * 512:q0 + (nn + 1) * 512],
                                start=True,
                                stop=True,
                            )
                        p = p_pm.tile([128, 1024], bf16, tag="pm", name="pm")
                        nc.scalar.activation(out=p, in_=sT, func=Exp, scale=SCALE)
                        pm = p_pm.tile([128, 1024], bf16, tag="pm", name="pm")
                        nc.vector.tensor_mul(
                            pm, p, keep_sl(kc, q0, q0 + 1024)
                        )
                        pending.append((kc, pm))
                        if len(pending) > 2:
                            consume(*pending.pop(0))
                    for item in pending:
                        consume(*item)
                    for qq in range(2):
                        # 1/d via exp(-ln(d)) on ACT: frees the PSUM
                        # accumulators fast and keeps DVE reciprocal (which
                        # measures ~6 cyc/elem) off the critical path.
                        lnd = p_sm.tile([128, 512], f32, tag="lnd", name="lnd")
                        nc.scalar.activation(out=lnd, in_=d_ps[qq], func=Ln)
                        rdb = p_sm.tile([128, 512], f32, tag="rdb", name="rdb")
                        nc.scalar.activation(out=rdb, in_=lnd, func=Exp, scale=-1.0)
                        nc.vector.tensor_mul(
                            oT_sb[h][:, q0 + qq * 512:q0 + (qq + 1) * 512],
                            o_ps[qq],
                            rdb,
                        )

            # --- output projection (partial; host adds the two groups + bias) ---
            for sc in range(NSC):
                ps = pp_big.tile([128, 1024], f32, tag="ppbig", name="ppbig")
                for h in range(HPG):
                    for nn in range(2):
                        nc.tensor.matmul(
                            ps[:, nn * 512:(nn + 1) * 512],
                            lhsT=oT_sb[h][:, sc * 128:(sc + 1) * 128],
                            rhs=outw_sb[h][:, nn * 512:(nn + 1) * 512],
                            start=(h == 0),
                            stop=(h == HPG - 1),
                        )
                osb = p_sm.tile([128, 1024], f32, tag="osb", name="osb")
                nc.vector.tensor_copy(osb, ps)
                nc.sync.dma_start(out=out[sc * 128:(sc + 1) * 128, :], in_=osb)

    _split_waits(nc, mybir, maxw=1)
    return nc


def _prep_core_inputs(x, attn_mask, qkv_w, qkv_b, q_w, q_b, k_w, k_b, v_w, v_b,
                      out_w):
    """Host-side: fold projections, shard, pre-transpose/tile, cast."""
    f = np.float32
    x = np.asarray(x, f)
    qkv_w = np.asarray(qkv_w, f)
    qkv_b = np.asarray(qkv_b, f)
    Ws = {}
    bs = {}
    for i, (w, b) in enumerate(((q_w, q_b), (k_w, k_b), (v_w, v_b))):
        w = np.asarray(w, f)
        b = np.asarray(b, f)
        sl = slice(i * D, (i + 1) * D)
        Ws[i] = w @ qkv_w[sl]              # [D, D] effective
        bs[i] = b + w @ qkv_b[sl]          # [D]
    out_wT = np.ascontiguousarray(np.asarray(out_w, f).T)  # [D(hd), D(model)]

    keepT = (np.asarray(attn_mask).T == 0).astype(BF16)    # [k, q]
    keepT_t = np.ascontiguousarray(keepT).reshape(NKC, 128, S)

    xT_all = []
    for b_i in range(B):
        xb = np.ascontiguousarray(x[b_i].T.astype(BF16))   # [D, S]
        xT_all.append(xb.reshape(ND, 128, S))

    maps = []
    for c in range(8):
        b_i = c % B
        g = c // B
        sl = slice(g * GD, (g + 1) * GD)
        m = {
            "xT": xT_all[b_i],
            "wqT": np.ascontiguousarray(Ws[0][sl].T.astype(BF16)).reshape(ND, 128, GD),
            "wkT": np.ascontiguousarray(Ws[1][sl].T.astype(BF16)).reshape(ND, 128, GD),
            "wvT": np.ascontiguousarray(Ws[2][sl].T.astype(BF16)).reshape(ND, 128, GD),
            "bq": np.ascontiguousarray(bs[0][sl].reshape(HPG, 128).T.astype(f)),
            "bk": np.ascontiguousarray(bs[1][sl].reshape(HPG, 128).T.astype(f)),
            "bvrow": bs[2][sl].astype(BF16).reshape(1, GD),
            "outwT": np.ascontiguousarray(out_wT[sl].astype(BF16)).reshape(HPG, 128, D),
            "keepT": keepT_t,
        }
        maps.append(m)
    return maps


def kernel(x, attn_mask, qkv_w, qkv_b, q_w, q_b, k_w, k_b, v_w, v_b,
           out_w, out_b, _trace=False):
    _install_ntff_hook_shim()
    from concourse.bass_utils import run_bass_kernel_spmd

    in_maps = _prep_core_inputs(
        x, attn_mask, qkv_w, qkv_b, q_w, q_b, k_w, k_b, v_w, v_b, out_w
    )
    use_vbias = bool(np.any(np.asarray(in_maps[0]["bvrow"], np.float32) != 0))
    key = ("nc", use_vbias)
    if key not in _cached:
        _cached[key] = _build_program(use_vbias=use_vbias)
    nc = _cached[key]
    core_ids = list(range(8))
    try:
        res = run_bass_kernel_spmd(nc, in_maps, core_ids, trace=_trace)
    except Exception:
        # transient NRT device wedge recovers on retry
        res = run_bass_kernel_spmd(nc, in_maps, core_ids, trace=_trace)
    _cached["last_result"] = res

    out_b = np.asarray(out_b, np.float32)
    full = np.empty((B, S, D), np.float32)
    for b_i in range(B):
        full[b_i] = (
            res.results[b_i]["out"] + res.results[b_i + B]["out"] + out_b
        )
    return full



# revision 3
# speedup vs baseline: 1.1774x; 1.1774x over previous
"""Multi-head self-attention Trainium2 kernel (8 NeuronCores).

Problem: B=4, S=2048, D=1024, H=8 heads (HD=128).
  qkv = x @ qkv_w.T + qkv_b ; q,k,v = split(qkv)
  q = (q @ q_w.T + q_b)  (same k, v) -> [B,H,S,HD]
  scores = q k^T * HD^-0.5, masked softmax (attn_mask==1 -> -inf), o = attn @ v
  out = o @ out_w.T + out_b

Sharding: 8 cores = 4 batches x 2 head-groups (4 heads each).
Core c: batch b = c % 4, head-group g = c // 4.

Host-side algebraic folding: the qkv projection and per-stream q/k/v
projections are composed into single effective weights (W_eff = w @
qkv_w_slice).  The out-projection is row-parallel across head-groups; the
two partial outputs per batch are summed on host with out_b.

Device flow per core (fp32 PSUM accumulation everywhere):
  q/k projections in fp8e4 DoubleRow (K=256 per matmul): x and W_eff are
    pre-scaled by 8 / 512 into e4m3 on host; the PSUM result (4096x) is
    descaled by the ACT identity that moves it to SBUF.  Softmax noise from
    fp8 q/k is ~1% on attention weights and averages out in o.
  v[S, 4*HD] in bf16 (v feeds o directly: fp8 would cost ~3% output error)
  per head, per q-half (1024 q), software-pipelined 2 chunks deep:
    for kc in 16 k-chunks:
      sT = kT_h[:,kc]^T @ qT_h        [128 k, 1024 q]   (PE -> PSUM f32)
      p  = exp(SCALE * sT)            (ACT -> bf16 SBUF)
      pm = p * keepT[kc]              (DVE; keep = attn_mask.T == 0)
      oT += v[kc]^T-as-lhsT @ pm      -> oT[HD, q]      (PE, PSUM accum)
      pair_j = pm[2j] + pm[2j+1]      (DVE/GpSimd pre-reduction)
      dB += ones^T @ pair_j           16 instead of 32 ones-matmuls
    oT_sb = oT * exp(-ln(dB))         softmax normalization (ACT+DVE -> bf16)
  out_partial[s,:] = sum_h oT_h[:,s_chunk]^T @ outwT_h   (+host bias/sum)
  The half-0 out-projection chunks interleave into half-1 attention so the
  output DMA spreads instead of tailing.
"""

import os
import sys
import types

sys.path.insert(0, "/opt/trn_rl_repo")

import numpy as np
import ml_dtypes

BF16 = ml_dtypes.bfloat16
F8E4 = ml_dtypes.float8_e4m3  # TRN fp8e4: max normal 240

B, S, D, H, HD = 4, 2048, 1024, 8, 128
HG = 2           # head groups
HPG = H // HG    # heads per group (4)
GD = HPG * HD    # dims per group (512)
SCALE = float(HD) ** -0.5
NKC = S // 128   # 16 k chunks
NSC = S // 128   # 16 s chunks
ND = D // 128    # 8 d chunks
NDP = ND // 2    # 4 d-pairs for DoubleRow

X_SCALE = 8.0
W_SCALE = 512.0
PROJ_DESCALE = 1.0 / (X_SCALE * W_SCALE)

# tuning flags
USE_DR = os.environ.get("K_USE_DR", "1") == "1"       # fp8 DoubleRow qk-proj
GPS_PAIRS = int(os.environ.get("K_GPS_PAIRS", "3"))   # denom pairs on GpSimd
OUT_BF16 = os.environ.get("K_OUT_BF16", "1") == "1"

_cached = {}


def _install_ntff_hook_shim():
    """The agent image's antenv lacks axon_hooks; shim it so trace works."""
    if "antenv.axon_hooks" in sys.modules:
        return
    try:
        import trn_agent_boot.trn_boot as _tb

        _hook = _tb._ntff_profile_via_ctypes("/opt/axon/libaxon_pjrt.so")
    except Exception:
        _hook = None
    _m = types.ModuleType("antenv.axon_hooks")
    _m.get_axon_ntff_profile_hook = lambda: _hook
    sys.modules["antenv.axon_hooks"] = _m


def _split_waits(nc, mybir, maxw=1):
    """Walrus in this image allows only one sync wait per instruction;
    hoist extra waits onto preceding NoOps on the same engine."""
    n_new = 0
    for fn in nc.m.functions:
        for bb in fn.blocks:
            newlist = []
            for inst in bb.instructions:
                si = inst.sync_info
                if si is not None and si.on_wait is not None and len(si.on_wait) > maxw:
                    waits = list(si.on_wait)
                    extra, keep = waits[:-maxw], waits[-maxw:]
                    while extra:
                        chunk, extra = extra[:maxw], extra[maxw:]
                        nop = mybir.InstNoOp(name=f"I-waitsplit-{nc.next_id()}")
                        nop.engine = inst.engine
                        nop.sync_info = mybir.SyncInfo(on_wait=chunk, on_update=[])
                        newlist.append(nop)
                        n_new += 1
                    si.on_wait = keep
                newlist.append(inst)
            bb.instructions = newlist
    return n_new


def _build_program(use_vbias=False, use_dr=USE_DR, gps_pairs=GPS_PAIRS,
                   out_bf16=OUT_BF16):
    import concourse.bass as bass
    import concourse.mybir as mybir
    import concourse.tile as tile

    f32 = mybir.dt.float32
    bf16 = mybir.dt.bfloat16
    fp8 = mybir.dt.float8e4
    Exp = mybir.ActivationFunctionType.Exp
    Ident = mybir.ActivationFunctionType.Identity
    Ln = mybir.ActivationFunctionType.Ln
    DR = mybir.MatmulPerfMode.DoubleRow

    nc = bass.Bass()

    # DRAM parameters (per-core shards, pre-tiled on host)
    if use_dr:
        x8 = nc.declare_dram_parameter("x8", [NDP, 128, 2, S], fp8, isOutput=False)
        wq8 = nc.declare_dram_parameter("wq8", [NDP, 128, 2, GD], fp8, isOutput=False)
        wk8 = nc.declare_dram_parameter("wk8", [NDP, 128, 2, GD], fp8, isOutput=False)
    else:
        wqT = nc.declare_dram_parameter("wqT", [ND, 128, GD], bf16, isOutput=False)
        wkT = nc.declare_dram_parameter("wkT", [ND, 128, GD], bf16, isOutput=False)
    xT = nc.declare_dram_parameter("xT", [ND, 128, S], bf16, isOutput=False)
    wvT = nc.declare_dram_parameter("wvT", [ND, 128, GD], bf16, isOutput=False)
    bq = nc.declare_dram_parameter("bq", [128, HPG], f32, isOutput=False)
    bk = nc.declare_dram_parameter("bk", [128, HPG], f32, isOutput=False)
    bvrow = nc.declare_dram_parameter("bvrow", [1, GD], bf16, isOutput=False)
    outwT = nc.declare_dram_parameter("outwT", [HPG, 128, D], bf16, isOutput=False)
    keepT = nc.declare_dram_parameter("keepT", [NKC, 128, S], bf16, isOutput=False)
    out_dt = bf16 if out_bf16 else f32
    out = nc.declare_dram_parameter("out", [S, D], out_dt, isOutput=True)

    with tile.TileContext(nc) as tc:
        import contextlib

        with contextlib.ExitStack() as ctx:
            # --- pools ---
            # big2k rotation (4KB slots): x8(4) + xT(8) + keep(0..3), then
            # keep(4..7) reuse the x8 slots after qk-proj and keep(8..15)
            # the xT slots after v-proj.
            p_big = ctx.enter_context(tc.tile_pool(name="big2k", bufs=16))
            p_pers = ctx.enter_context(tc.tile_pool(name="pers", bufs=1))
            p_pm = ctx.enter_context(tc.tile_pool(name="pm", bufs=10))
            p_acc = ctx.enter_context(tc.tile_pool(name="acc", bufs=6))
            p_sm = ctx.enter_context(tc.tile_pool(name="small", bufs=2))
            pp_big = ctx.enter_context(tc.tile_pool(name="ppbig", bufs=2, space="PSUM"))
            pp_o = ctx.enter_context(tc.tile_pool(name="ppo", bufs=4, space="PSUM"))

            # --- constants ---
            ones128 = p_pers.tile([128, 128], bf16, tag="ones128", name="ones128")
            nc.vector.memset(ones128, 1.0)

            # --- DMAs, first-needed first ---
            x8_tiles = []
            w8 = {}
            if use_dr:
                for dp in range(NDP):
                    t = p_pers.tile([128, 2, GD], fp8, tag=f"wq8{dp}", name=f"wq8{dp}")
                    nc.sync.dma_start(out=t, in_=wq8[dp])
                    w8[("q", dp)] = t
                    t = p_big.tile([128, 2, S], fp8, tag="big2k", name="big2k")
                    nc.sync.dma_start(out=t, in_=x8[dp])
                    x8_tiles.append(t)
                for dp in range(NDP):
                    t = p_pers.tile([128, 2, GD], fp8, tag=f"wk8{dp}", name=f"wk8{dp}")
                    nc.sync.dma_start(out=t, in_=wk8[dp])
                    w8[("k", dp)] = t
            else:
                for d in range(ND):
                    t = p_pers.tile([128, GD], bf16, tag=f"wq{d}", name=f"wq{d}")
                    nc.sync.dma_start(out=t, in_=wqT[d])
                    w8[("q", d)] = t
                for d in range(ND):
                    t = p_pers.tile([128, GD], bf16, tag=f"wk{d}", name=f"wk{d}")
                    nc.sync.dma_start(out=t, in_=wkT[d])
                    w8[("k", d)] = t

            bq_sb = p_pers.tile([128, HPG], f32, tag="bq", name="bq_sb")
            nc.sync.dma_start(out=bq_sb, in_=bq[:, :])
            bk_sb = p_pers.tile([128, HPG], f32, tag="bk", name="bk_sb")
            nc.sync.dma_start(out=bk_sb, in_=bk[:, :])

            xt_tiles = []
            wv_sb = []
            for d in range(ND):
                t = p_big.tile([128, S], bf16, tag="big2k", name="big2k")
                nc.sync.dma_start(out=t, in_=xT[d])
                xt_tiles.append(t)
                t = p_pers.tile([128, GD], bf16, tag=f"wv{d}", name=f"wv{d}")
                nc.sync.dma_start(out=t, in_=wvT[d])
                wv_sb.append(t)

            bv_sb = None
            if use_vbias:
                bv_sb = p_pers.tile([1, GD], bf16, tag="bv", name="bv_sb")
                nc.sync.dma_start(out=bv_sb, in_=bvrow[:, :])

            outw_sb = []
            for h in range(HPG):
                t = p_pers.tile([128, D], bf16, tag=f"outw{h}", name=f"outw{h}")
                nc.sync.dma_start(out=t, in_=outwT[h])
                outw_sb.append(t)

            keep_tiles = [None] * NKC
            for kc in range(4):
                t = p_big.tile([128, S], bf16, tag="big2k", name="big2k")
                nc.sync.dma_start(out=t, in_=keepT[kc])
                keep_tiles[kc] = t

            def keep_sl(kc, lo, hi):
                return keep_tiles[kc][:, lo:hi]

            # --- q/k projections ---
            qT_sb = [p_pers.tile([128, S], bf16, tag=f"qT{h}", name=f"qT{h}") for h in range(HPG)]
            kT_sb = [p_pers.tile([128, S], bf16, tag=f"kT{h}", name=f"kT{h}") for h in range(HPG)]

            if use_dr:
                # per (head, stream): 4 psum quarters accumulated over 4
                # d-pairs; lhsT is the same for the 4 quarters of a dp step,
                # so weights load once per 4 matmuls.
                for h in range(HPG):
                    for sname, dst, bias in (("q", qT_sb, bq_sb), ("k", kT_sb, bk_sb)):
                        pss = [
                            pp_o.tile([128, 512], f32, tag="ppo", name="ppo")
                            for _ in range(4)
                        ]
                        for dp in range(NDP):
                            lhs = w8[(sname, dp)][:, :, h * 128:(h + 1) * 128]
                            for qu in range(4):
                                nc.tensor.matmul(
                                    pss[qu],
                                    lhsT=lhs,
                                    rhs=x8_tiles[dp][:, :, qu * 512:(qu + 1) * 512],
                                    start=(dp == 0),
                                    stop=(dp == NDP - 1),
                                    perf_mode=DR,
                                )
                        for qu in range(4):
                            nc.scalar.activation(
                                out=dst[h][:, qu * 512:(qu + 1) * 512],
                                in_=pss[qu],
                                func=Ident,
                                bias=bias[:, h:h + 1],
                                scale=PROJ_DESCALE,
                            )
            else:
                for h in range(HPG):
                    for sname, dst, bias in (("q", qT_sb, bq_sb), ("k", kT_sb, bk_sb)):
                        pss = [
                            pp_o.tile([128, 512], f32, tag="ppo", name="ppo")
                            for _ in range(4)
                        ]
                        for d in range(ND):
                            lhs = w8[(sname, d)][:, h * 128:(h + 1) * 128]
                            for qu in range(4):
                                nc.tensor.matmul(
                                    pss[qu],
                                    lhsT=lhs,
                                    rhs=xt_tiles[d][:, qu * 512:(qu + 1) * 512],
                                    start=(d == 0),
                                    stop=(d == ND - 1),
                                )
                        for qu in range(4):
                            nc.scalar.activation(
                                out=dst[h][:, qu * 512:(qu + 1) * 512],
                                in_=pss[qu],
                                func=Ident,
                                bias=bias[:, h:h + 1],
                            )

            # keep(4..7) into the freed x8 slots (or into the rotation after
            # the bf16 path's first four reuses)
            for kc in range(4, 8):
                t = p_big.tile([128, S], bf16, tag="big2k", name="big2k")
                nc.sync.dma_start(out=t, in_=keepT[kc])
                keep_tiles[kc] = t

            # --- v projection (bf16; fp8 v would cost ~3% output error) ---
            v_sb = [p_pers.tile([128, GD], bf16, tag=f"v{sc}", name=f"v{sc}") for sc in range(NSC)]
            for sc in range(NSC):
                ps = pp_o.tile([128, GD], f32, tag="ppo", name="ppo")
                for d in range(ND):
                    nc.tensor.matmul(
                        ps,
                        lhsT=xt_tiles[d][:, sc * 128:(sc + 1) * 128],
                        rhs=wv_sb[d],
                        start=(d == 0),
                        stop=(d == ND - 1) and not use_vbias,
                    )
                if use_vbias:
                    nc.tensor.matmul(
                        ps,
                        lhsT=ones128[0:1, :],
                        rhs=bv_sb,
                        start=False,
                        stop=True,
                    )
                nc.vector.tensor_copy(v_sb[sc], ps)

            # keep(8..15) into the freed xT slots
            for kc in range(8, NKC):
                t = p_big.tile([128, S], bf16, tag="big2k", name="big2k")
                nc.sync.dma_start(out=t, in_=keepT[kc])
                keep_tiles[kc] = t

            # --- attention (half-major so half-0 out-projection can
            # interleave into half-1) + out-projection ---
            oT_sb = [p_pers.tile([128, S], bf16, tag=f"oT{h}", name=f"oT{h}") for h in range(HPG)]

            osb_dt = bf16 if out_bf16 else f32

            def out_proj(sc):
                ps = pp_big.tile([128, 1024], f32, tag="ppbig", name="ppbig")
                for h in range(HPG):
                    for nn in range(2):
                        nc.tensor.matmul(
                            ps[:, nn * 512:(nn + 1) * 512],
                            lhsT=oT_sb[h][:, sc * 128:(sc + 1) * 128],
                            rhs=outw_sb[h][:, nn * 512:(nn + 1) * 512],
                            start=(h == 0),
                            stop=(h == HPG - 1),
                        )
                osb = p_sm.tile([128, 1024], osb_dt, tag="osb", name="osb")
                nc.vector.tensor_copy(osb, ps)
                nc.sync.dma_start(out=out[sc * 128:(sc + 1) * 128, :], in_=osb)

            def attention_hh(h, half, interleave):
                """interleave: list of sc chunks to out-project, spread
                through this hh's k-chunk loop."""
                q0 = half * 1024
                o_ps = [pp_o.tile([128, 512], f32, tag="ppo", name="ppo") for _ in range(2)]
                d_ps = [pp_o.tile([128, 512], f32, tag="ppo", name="ppo") for _ in range(2)]

                def consume(kc, pm):
                    for qq in range(2):
                        nc.tensor.matmul(
                            o_ps[qq],
                            lhsT=v_sb[kc][:, h * 128:(h + 1) * 128],
                            rhs=pm[:, qq * 512:(qq + 1) * 512],
                            start=(kc == 0),
                            stop=(kc == NKC - 1),
                        )

                def d_mm(pr, pacc):
                    for qq in range(2):
                        nc.tensor.matmul(
                            d_ps[qq],
                            lhsT=ones128,
                            rhs=pacc[:, qq * 512:(qq + 1) * 512],
                            start=(pr == 0),
                            stop=(pr == NKC // 2 - 1),
                        )

                pending = []      # [(kc, pm)] — 2-stage consume delay
                pairs = [None] * (NKC // 2)   # accumulated pm pairs
                d_emit = []       # pairs ready to ones-matmul
                pm_even = None
                for kc in range(NKC):
                    sT = pp_big.tile([128, 1024], f32, tag="ppbig", name="ppbig")
                    for nn in range(2):
                        nc.tensor.matmul(
                            sT[:, nn * 512:(nn + 1) * 512],
                            lhsT=kT_sb[h][:, kc * 128:(kc + 1) * 128],
                            rhs=qT_sb[h][:, q0 + nn * 512:q0 + (nn + 1) * 512],
                            start=True,
                            stop=True,
                        )
                    p = p_pm.tile([128, 1024], bf16, tag="pm", name="pm")
                    nc.scalar.activation(out=p, in_=sT, func=Exp, scale=SCALE)
                    pm = p_pm.tile([128, 1024], bf16, tag="pm", name="pm")
                    nc.vector.tensor_mul(pm, p, keep_sl(kc, q0, q0 + 1024))

                    if kc % 2 == 0:
                        pm_even = pm
                    else:
                        pr = kc // 2
                        pacc = p_acc.tile([128, 1024], bf16, tag="acc", name="acc")
                        eng = nc.gpsimd if pr < gps_pairs else nc.vector
                        eng.tensor_add(pacc, pm_even, pm)
                        pairs[pr] = pacc
                        d_emit.append(pr)

                    pending.append((kc, pm))
                    if len(pending) > 2:
                        consume(*pending.pop(0))
                    # ones-matmul a pair two k-chunks after it was formed so
                    # the PE never waits on the DVE/GpSimd adds
                    if d_emit and d_emit[0] <= (kc - 3) // 2:
                        pr = d_emit.pop(0)
                        d_mm(pr, pairs[pr])
                for item in pending:
                    consume(*item)
                for pr in d_emit:
                    d_mm(pr, pairs[pr])

                if interleave:
                    out_proj(interleave.pop(0))

                for qq in range(2):
                    # 1/d via exp(-ln(d)) on ACT: frees the PSUM accumulators
                    # fast and keeps DVE reciprocal off the critical path.
                    lnd = p_sm.tile([128, 512], f32, tag="lnd", name="lnd")
                    nc.scalar.activation(out=lnd, in_=d_ps[qq], func=Ln)
                    rdb = p_sm.tile([128, 512], f32, tag="rdb", name="rdb")
                    nc.scalar.activation(out=rdb, in_=lnd, func=Exp, scale=-1.0)
                    nc.vector.tensor_mul(
                        oT_sb[h][:, q0 + qq * 512:q0 + (qq + 1) * 512],
                        o_ps[qq],
                        rdb,
                    )

                if interleave:
                    out_proj(interleave.pop(0))

            for h in range(HPG):
                attention_hh(h, 0, [])
            sc_queue = list(range(8))
            for h in range(HPG):
                attention_hh(h, 1, [sc_queue.pop(0), sc_queue.pop(0)])
            for sc in range(8, NSC):
                out_proj(sc)

    _split_waits(nc, mybir, maxw=1)
    return nc


def _prep_core_inputs(x, attn_mask, qkv_w, qkv_b, q_w, q_b, k_w, k_b, v_w, v_b,
                      out_w, use_dr=USE_DR):
    """Host-side: fold projections, shard, pre-transpose/tile, cast."""
    f = np.float32
    x = np.asarray(x, f)
    qkv_w = np.asarray(qkv_w, f)
    qkv_b = np.asarray(qkv_b, f)
    Ws = {}
    bs = {}
    for i, (w, b) in enumerate(((q_w, q_b), (k_w, k_b), (v_w, v_b))):
        w = np.asarray(w, f)
        b = np.asarray(b, f)
        sl = slice(i * D, (i + 1) * D)
        Ws[i] = w @ qkv_w[sl]              # [D, D] effective
        bs[i] = b + w @ qkv_b[sl]          # [D]
    out_wT = np.ascontiguousarray(np.asarray(out_w, f).T)  # [D(hd), D(model)]

    keepT = (np.asarray(attn_mask).T == 0).astype(BF16)    # [k, q]
    keepT_t = np.ascontiguousarray(keepT).reshape(NKC, 128, S)

    def dr_pack(mat_T, scale):
        # mat_T: [D, N] (contraction-major) -> [NDP, 128, 2, N] e4m3 * scale
        m = (mat_T * scale).reshape(NDP, 2, 128, -1).transpose(0, 2, 1, 3)
        return np.ascontiguousarray(m.astype(F8E4))

    xT_all = []
    x8_all = []
    for b_i in range(B):
        xb = np.ascontiguousarray(x[b_i].T)                # [D, S] f32
        xT_all.append(np.ascontiguousarray(xb.astype(BF16)).reshape(ND, 128, S))
        if use_dr:
            x8_all.append(dr_pack(xb, X_SCALE))

    maps = []
    for c in range(8):
        b_i = c % B
        g = c // B
        sl = slice(g * GD, (g + 1) * GD)
        m = {
            "xT": xT_all[b_i],
            "wvT": np.ascontiguousarray(Ws[2][sl].T.astype(BF16)).reshape(ND, 128, GD),
            "bq": np.ascontiguousarray(bs[0][sl].reshape(HPG, 128).T.astype(f)),
            "bk": np.ascontiguousarray(bs[1][sl].reshape(HPG, 128).T.astype(f)),
            "bvrow": bs[2][sl].astype(BF16).reshape(1, GD),
            "outwT": np.ascontiguousarray(out_wT[sl].astype(BF16)).reshape(HPG, 128, D),
            "keepT": keepT_t,
        }
        if use_dr:
            m["x8"] = x8_all[b_i]
            m["wq8"] = dr_pack(np.ascontiguousarray(Ws[0][sl].T), W_SCALE)
            m["wk8"] = dr_pack(np.ascontiguousarray(Ws[1][sl].T), W_SCALE)
        else:
            m["wqT"] = np.ascontiguousarray(Ws[0][sl].T.astype(BF16)).reshape(ND, 128, GD)
            m["wkT"] = np.ascontiguousarray(Ws[1][sl].T.astype(BF16)).reshape(ND, 128, GD)
        maps.append(m)
    return maps


def kernel(x, attn_mask, qkv_w, qkv_b, q_w, q_b, k_w, k_b, v_w, v_b,
           out_w, out_b, _trace=False):
    _install_ntff_hook_shim()
    from concourse.bass_utils import run_bass_kernel_spmd

    in_maps = _prep_core_inputs(
        x, attn_mask, qkv_w, qkv_b, q_w, q_b, k_w, k_b, v_w, v_b, out_w
    )
    use_vbias = bool(np.any(np.asarray(in_maps[0]["bvrow"], np.float32) != 0))
    key = ("nc", use_vbias)
    if key not in _cached:
        _cached[key] = _build_program(use_vbias=use_vbias)
    nc = _cached[key]
    core_ids = list(range(8))
    try:
        res = run_bass_kernel_spmd(nc, in_maps, core_ids, trace=_trace)
    except Exception:
        # transient NRT device wedge recovers on retry
        res = run_bass_kernel_spmd(nc, in_maps, core_ids, trace=_trace)
    _cached["last_result"] = res

    out_b = np.asarray(out_b, np.float32)
    full = np.empty((B, S, D), np.float32)
    for b_i in range(B):
        full[b_i] = (
            res.results[b_i]["out"].astype(np.float32)
            + res.results[b_i + B]["out"].astype(np.float32)
            + out_b
        )
    return full


# revision 8
# speedup vs baseline: 1.2636x; 1.0732x over previous
"""Multi-head self-attention Trainium2 kernel (8 NeuronCores).

Problem: B=4, S=2048, D=1024, H=8 heads (HD=128).
  qkv = x @ qkv_w.T + qkv_b ; q,k,v = split(qkv)
  q = (q @ q_w.T + q_b)  (same k, v) -> [B,H,S,HD]
  scores = q k^T * HD^-0.5, masked softmax (attn_mask==1 -> -inf), o = attn @ v
  out = o @ out_w.T + out_b

Sharding: 8 cores = 4 batches x 2 head-groups (4 heads each).
Core c: batch b = c % 4, head-group g = c // 4.

Host-side algebraic folding: the qkv projection and per-stream q/k/v
projections are composed into single effective weights (W_eff = w @
qkv_w_slice).  The out-projection is row-parallel across head-groups; the
two partial outputs per batch are summed on host with out_b.

Device flow per core (fp32 PSUM accumulation everywhere):
  q/k projections in fp8e4 DoubleRow (K=256 per matmul): x and W_eff are
    pre-scaled by 8 / 512 into e4m3 on host; the PSUM result (4096x) is
    descaled by the ACT identity that moves it to SBUF.  Softmax noise from
    fp8 q/k is ~1% on attention weights and averages out in o.
  v[S, 4*HD] in bf16 (v feeds o directly: fp8 would cost ~3% output error)
  per head, per q-half (1024 q), software-pipelined 2 chunks deep:
    for kc in 16 k-chunks:
      sT = kT_h[:,kc]^T @ qT_h        [128 k, 1024 q]   (PE -> PSUM f32)
      p  = exp(SCALE * sT)            (ACT -> bf16 SBUF)
      pm = p * keepT[kc]              (DVE; keep = attn_mask.T == 0)
      oT += v[kc]^T-as-lhsT @ pm      -> oT[HD, q]      (PE, PSUM accum)
      pair_j = pm[2j] + pm[2j+1]      (DVE/GpSimd pre-reduction)
      dB += ones^T @ pair_j           16 instead of 32 ones-matmuls
    oT_sb = oT * exp(-ln(dB))         softmax normalization (ACT+DVE -> bf16)
  out_partial[s,:] = sum_h oT_h[:,s_chunk]^T @ outwT_h   (+host bias/sum)
  The half-0 out-projection chunks interleave into half-1 attention so the
  output DMA spreads instead of tailing.
"""

import os
import sys
import types

sys.path.insert(0, "/opt/trn_rl_repo")

import numpy as np
import ml_dtypes

BF16 = ml_dtypes.bfloat16
F8E4 = ml_dtypes.float8_e4m3  # TRN fp8e4: max normal 240

B, S, D, H, HD = 4, 2048, 1024, 8, 128
HG = 2           # head groups
HPG = H // HG    # heads per group (4)
GD = HPG * HD    # dims per group (512)
SCALE = float(HD) ** -0.5
NKC = S // 128   # 16 k chunks
NSC = S // 128   # 16 s chunks
ND = D // 128    # 8 d chunks
NDP = ND // 2    # 4 d-pairs for DoubleRow

X_SCALE = 8.0
W_SCALE = 512.0
PROJ_DESCALE = 1.0 / (X_SCALE * W_SCALE)

# tuning flags
USE_DR = os.environ.get("K_USE_DR", "1") == "1"       # fp8 DoubleRow qk-proj
# GpSimd offload measured net-negative: its SBUF port is an exclusive lock
# shared with DVE, and DVE tensor ops under a concurrent GpSimd ADD run ~2x
# slower (1348ns vs 602ns for the [128,1024] pm multiply).
GPS_PAIRS = int(os.environ.get("K_GPS_PAIRS", "0"))   # denom pairs on GpSimd
OUT_BF16 = os.environ.get("K_OUT_BF16", "1") == "1"

_cached = {}


def _install_ntff_hook_shim():
    """The agent image's antenv lacks axon_hooks; shim it so trace works."""
    if "antenv.axon_hooks" in sys.modules:
        return
    try:
        import trn_agent_boot.trn_boot as _tb

        _hook = _tb._ntff_profile_via_ctypes("/opt/axon/libaxon_pjrt.so")
    except Exception:
        _hook = None
    _m = types.ModuleType("antenv.axon_hooks")
    _m.get_axon_ntff_profile_hook = lambda: _hook
    sys.modules["antenv.axon_hooks"] = _m


def _split_waits(nc, mybir, maxw=1):
    """Walrus in this image allows only one sync wait per instruction;
    hoist extra waits onto preceding NoOps on the same engine."""
    n_new = 0
    for fn in nc.m.functions:
        for bb in fn.blocks:
            newlist = []
            for inst in bb.instructions:
                si = inst.sync_info
                if si is not None and si.on_wait is not None and len(si.on_wait) > maxw:
                    waits = list(si.on_wait)
                    extra, keep = waits[:-maxw], waits[-maxw:]
                    while extra:
                        chunk, extra = extra[:maxw], extra[maxw:]
                        nop = mybir.InstNoOp(name=f"I-waitsplit-{nc.next_id()}")
                        nop.engine = inst.engine
                        nop.sync_info = mybir.SyncInfo(on_wait=chunk, on_update=[])
                        newlist.append(nop)
                        n_new += 1
                    si.on_wait = keep
                newlist.append(inst)
            bb.instructions = newlist
    return n_new


def _build_program(use_vbias=False, use_dr=USE_DR, gps_pairs=GPS_PAIRS,
                   out_bf16=OUT_BF16):
    import concourse.bass as bass
    import concourse.mybir as mybir
    import concourse.tile as tile

    f32 = mybir.dt.float32
    bf16 = mybir.dt.bfloat16
    fp8 = mybir.dt.float8e4
    Exp = mybir.ActivationFunctionType.Exp
    Ident = mybir.ActivationFunctionType.Identity
    Ln = mybir.ActivationFunctionType.Ln
    DR = mybir.MatmulPerfMode.DoubleRow

    nc = bass.Bass()

    # DRAM parameters (per-core shards, pre-tiled on host)
    if use_dr:
        x8 = nc.declare_dram_parameter("x8", [NDP, 128, 2, S], fp8, isOutput=False)
        wq8 = nc.declare_dram_parameter("wq8", [NDP, 128, 2, GD], fp8, isOutput=False)
        wk8 = nc.declare_dram_parameter("wk8", [NDP, 128, 2, GD], fp8, isOutput=False)
    else:
        wqT = nc.declare_dram_parameter("wqT", [ND, 128, GD], bf16, isOutput=False)
        wkT = nc.declare_dram_parameter("wkT", [ND, 128, GD], bf16, isOutput=False)
    xT = nc.declare_dram_parameter("xT", [ND, 128, S], bf16, isOutput=False)
    wvT = nc.declare_dram_parameter("wvT", [ND, 128, GD], bf16, isOutput=False)
    bq = nc.declare_dram_parameter("bq", [128, HPG], f32, isOutput=False)
    bk = nc.declare_dram_parameter("bk", [128, HPG], f32, isOutput=False)
    bvrow = nc.declare_dram_parameter("bvrow", [1, GD], bf16, isOutput=False)
    outwT = nc.declare_dram_parameter("outwT", [HPG, 128, D], bf16, isOutput=False)
    keepT = nc.declare_dram_parameter("keepT", [NKC, 128, S], bf16, isOutput=False)
    out_dt = bf16 if out_bf16 else f32
    out = nc.declare_dram_parameter("out", [S, D], out_dt, isOutput=True)

    with tile.TileContext(nc) as tc:
        import contextlib

        with contextlib.ExitStack() as ctx:
            # --- pools ---
            # big2k rotation (4KB slots): x8(4) + xT(8) + keep(0..3), then
            # keep(4..7) reuse the x8 slots after qk-proj and keep(8..15)
            # the xT slots after v-proj.
            p_big = ctx.enter_context(tc.tile_pool(name="big2k", bufs=16))
            p_pers = ctx.enter_context(tc.tile_pool(name="pers", bufs=1))
            p_pm = ctx.enter_context(tc.tile_pool(name="pm", bufs=10))
            p_acc = ctx.enter_context(tc.tile_pool(name="acc", bufs=6))
            p_sm = ctx.enter_context(tc.tile_pool(name="small", bufs=2))
            pp_big = ctx.enter_context(tc.tile_pool(name="ppbig", bufs=2, space="PSUM"))
            pp_o = ctx.enter_context(tc.tile_pool(name="ppo", bufs=4, space="PSUM"))

            # --- constants ---
            ones128 = p_pers.tile([128, 128], bf16, tag="ones128", name="ones128")
            nc.vector.memset(ones128, 1.0)

            # --- DMAs, first-needed first ---
            x8_tiles = []
            w8 = {}
            if use_dr:
                # weights first (small), then x8 in qu-quarters so the first
                # (stream, quarter) projection group starts after ~256KB
                for dp in range(NDP):
                    t = p_pers.tile([128, 2, GD], fp8, tag=f"wq8{dp}", name=f"wq8{dp}")
                    nc.sync.dma_start(out=t, in_=wq8[dp])
                    w8[("q", dp)] = t
                for dp in range(NDP):
                    t = p_big.tile([128, 2, S], fp8, tag="big2k", name="big2k")
                    x8_tiles.append(t)
                for qu in range(4):
                    for dp in range(NDP):
                        nc.sync.dma_start(
                            out=x8_tiles[dp][:, :, qu * 512:(qu + 1) * 512],
                            in_=x8[dp][:, :, qu * 512:(qu + 1) * 512],
                        )
                for dp in range(NDP):
                    t = p_pers.tile([128, 2, GD], fp8, tag=f"wk8{dp}", name=f"wk8{dp}")
                    nc.sync.dma_start(out=t, in_=wk8[dp])
                    w8[("k", dp)] = t
            else:
                for d in range(ND):
                    t = p_pers.tile([128, GD], bf16, tag=f"wq{d}", name=f"wq{d}")
                    nc.sync.dma_start(out=t, in_=wqT[d])
                    w8[("q", d)] = t
                for d in range(ND):
                    t = p_pers.tile([128, GD], bf16, tag=f"wk{d}", name=f"wk{d}")
                    nc.sync.dma_start(out=t, in_=wkT[d])
                    w8[("k", d)] = t

            bq_sb = p_pers.tile([128, HPG], f32, tag="bq", name="bq_sb")
            nc.sync.dma_start(out=bq_sb, in_=bq[:, :])
            bk_sb = p_pers.tile([128, HPG], f32, tag="bk", name="bk_sb")
            nc.sync.dma_start(out=bk_sb, in_=bk[:, :])

            xt_tiles = []
            wv_sb = []
            for d in range(ND):
                t = p_big.tile([128, S], bf16, tag="big2k", name="big2k")
                nc.sync.dma_start(out=t, in_=xT[d])
                xt_tiles.append(t)
                t = p_pers.tile([128, GD], bf16, tag=f"wv{d}", name=f"wv{d}")
                nc.sync.dma_start(out=t, in_=wvT[d])
                wv_sb.append(t)

            bv_sb = None
            if use_vbias:
                bv_sb = p_pers.tile([1, GD], bf16, tag="bv", name="bv_sb")
                nc.sync.dma_start(out=bv_sb, in_=bvrow[:, :])

            outw_sb = []
            for h in range(HPG):
                t = p_pers.tile([128, D], bf16, tag=f"outw{h}", name=f"outw{h}")
                nc.sync.dma_start(out=t, in_=outwT[h])
                outw_sb.append(t)

            keep_tiles = [None] * NKC
            for kc in range(4):
                t = p_big.tile([128, S], bf16, tag="big2k", name="big2k")
                nc.sync.dma_start(out=t, in_=keepT[kc])
                keep_tiles[kc] = t

            def keep_sl(kc, lo, hi):
                return keep_tiles[kc][:, lo:hi]

            # --- q/k projections ---
            qT_sb = [p_pers.tile([128, S], bf16, tag=f"qT{h}", name=f"qT{h}") for h in range(HPG)]
            kT_sb = [p_pers.tile([128, S], bf16, tag=f"kT{h}", name=f"kT{h}") for h in range(HPG)]

            if use_dr:
                # per (stream, quarter): 4 per-head psum accumulators over 4
                # d-pairs; group (s, qu) only needs the qu-quarter of x8, so
                # the first matmul gates on wq8[0] + one x8 quarter.
                for sname, dst, bias in (("q", qT_sb, bq_sb), ("k", kT_sb, bk_sb)):
                    for qu in range(4):
                        pss = [
                            pp_o.tile([128, 512], f32, tag="ppo", name="ppo")
                            for _ in range(HPG)
                        ]
                        for dp in range(NDP):
                            rhs = x8_tiles[dp][:, :, qu * 512:(qu + 1) * 512]
                            for h in range(HPG):
                                nc.tensor.matmul(
                                    pss[h],
                                    lhsT=w8[(sname, dp)][:, :, h * 128:(h + 1) * 128],
                                    rhs=rhs,
                                    start=(dp == 0),
                                    stop=(dp == NDP - 1),
                                    perf_mode=DR,
                                )
                        for h in range(HPG):
                            nc.scalar.activation(
                                out=dst[h][:, qu * 512:(qu + 1) * 512],
                                in_=pss[h],
                                func=Ident,
                                bias=bias[:, h:h + 1],
                                scale=PROJ_DESCALE,
                            )
            else:
                for h in range(HPG):
                    for sname, dst, bias in (("q", qT_sb, bq_sb), ("k", kT_sb, bk_sb)):
                        pss = [
                            pp_o.tile([128, 512], f32, tag="ppo", name="ppo")
                            for _ in range(4)
                        ]
                        for d in range(ND):
                            lhs = w8[(sname, d)][:, h * 128:(h + 1) * 128]
                            for qu in range(4):
                                nc.tensor.matmul(
                                    pss[qu],
                                    lhsT=lhs,
                                    rhs=xt_tiles[d][:, qu * 512:(qu + 1) * 512],
                                    start=(d == 0),
                                    stop=(d == ND - 1),
                                )
                        for qu in range(4):
                            nc.scalar.activation(
                                out=dst[h][:, qu * 512:(qu + 1) * 512],
                                in_=pss[qu],
                                func=Ident,
                                bias=bias[:, h:h + 1],
                            )

            # keep(4..7) into the freed x8 slots (or into the rotation after
            # the bf16 path's first four reuses)
            for kc in range(4, 8):
                t = p_big.tile([128, S], bf16, tag="big2k", name="big2k")
                nc.sync.dma_start(out=t, in_=keepT[kc])
                keep_tiles[kc] = t

            # --- v projection (bf16; fp8 v would cost ~3% output error) ---
            v_sb = [p_pers.tile([128, GD], bf16, tag=f"v{sc}", name=f"v{sc}") for sc in range(NSC)]
            for sc in range(NSC):
                ps = pp_o.tile([128, GD], f32, tag="ppo", name="ppo")
                for d in range(ND):
                    nc.tensor.matmul(
                        ps,
                        lhsT=xt_tiles[d][:, sc * 128:(sc + 1) * 128],
                        rhs=wv_sb[d],
                        start=(d == 0),
                        stop=(d == ND - 1) and not use_vbias,
                    )
                if use_vbias:
                    nc.tensor.matmul(
                        ps,
                        lhsT=ones128[0:1, :],
                        rhs=bv_sb,
                        start=False,
                        stop=True,
                    )
                nc.vector.tensor_copy(v_sb[sc], ps)

            # keep(8..15) into the freed xT slots
            for kc in range(8, NKC):
                t = p_big.tile([128, S], bf16, tag="big2k", name="big2k")
                nc.sync.dma_start(out=t, in_=keepT[kc])
                keep_tiles[kc] = t

            # --- attention (half-major so half-0 out-projection can
            # interleave into half-1) + out-projection ---
            oT_sb = [p_pers.tile([128, S], bf16, tag=f"oT{h}", name=f"oT{h}") for h in range(HPG)]

            osb_dt = bf16 if out_bf16 else f32

            def out_proj(sc):
                ps = pp_big.tile([128, 1024], f32, tag="ppbig", name="ppbig")
                for h in range(HPG):
                    for nn in range(2):
                        nc.tensor.matmul(
                            ps[:, nn * 512:(nn + 1) * 512],
                            lhsT=oT_sb[h][:, sc * 128:(sc + 1) * 128],
                            rhs=outw_sb[h][:, nn * 512:(nn + 1) * 512],
                            start=(h == 0),
                            stop=(h == HPG - 1),
                        )
                osb = p_sm.tile([128, 1024], osb_dt, tag="osb", name="osb")
                nc.vector.tensor_copy(osb, ps)
                nc.sync.dma_start(out=out[sc * 128:(sc + 1) * 128, :], in_=osb)

            def attention_hh(h, half, interleave, finish_prev=None):
                """interleave: sc chunks to out-project after this hh.
                finish_prev: the previous hh's deferred softmax-normalize,
                emitted after this hh's kc=2 so the boundary exps run
                back-to-back on ACT (emitting ln/exp between halves starved
                the sT rotation and stalled the PE ~1.3us, 4x per hh)."""
                q0 = half * 1024
                o_ps = [pp_o.tile([128, 512], f32, tag="ppo", name="ppo") for _ in range(2)]
                d_ps = [pp_o.tile([128, 512], f32, tag="ppo", name="ppo") for _ in range(2)]

                def consume(kc, pm):
                    for qq in range(2):
                        nc.tensor.matmul(
                            o_ps[qq],
                            lhsT=v_sb[kc][:, h * 128:(h + 1) * 128],
                            rhs=pm[:, qq * 512:(qq + 1) * 512],
                            start=(kc == 0),
                            stop=(kc == NKC - 1),
                        )

                def d_mm(pr, pacc):
                    for qq in range(2):
                        nc.tensor.matmul(
                            d_ps[qq],
                            lhsT=ones128,
                            rhs=pacc[:, qq * 512:(qq + 1) * 512],
                            start=(pr == 0),
                            stop=(pr == NKC // 2 - 1),
                        )

                pending = []      # [(kc, pm)] — 2-stage consume delay
                pairs = [None] * (NKC // 2)   # accumulated pm pairs
                d_emit = []       # pairs ready to ones-matmul
                pm_even = None
                for kc in range(NKC):
                    sT = pp_big.tile([128, 1024], f32, tag="ppbig", name="ppbig")
                    for nn in range(2):
                        nc.tensor.matmul(
                            sT[:, nn * 512:(nn + 1) * 512],
                            lhsT=kT_sb[h][:, kc * 128:(kc + 1) * 128],
                            rhs=qT_sb[h][:, q0 + nn * 512:q0 + (nn + 1) * 512],
                            start=True,
                            stop=True,
                        )
                    p = p_pm.tile([128, 1024], bf16, tag="pm", name="pm")
                    nc.scalar.activation(out=p, in_=sT, func=Exp, scale=SCALE)
                    pm = p_pm.tile([128, 1024], bf16, tag="pm", name="pm")
                    nc.vector.tensor_mul(pm, p, keep_sl(kc, q0, q0 + 1024))

                    if kc % 2 == 0:
                        pm_even = pm
                    else:
                        pr = kc // 2
                        pacc = p_acc.tile([128, 1024], bf16, tag="acc", name="acc")
                        eng = nc.gpsimd if pr < gps_pairs else nc.vector
                        eng.tensor_add(pacc, pm_even, pm)
                        pairs[pr] = pacc
                        d_emit.append(pr)

                    pending.append((kc, pm))
                    if len(pending) > 3:
                        consume(*pending.pop(0))
                    # ones-matmul a pair two k-chunks after it was formed so
                    # the PE never waits on the DVE adds
                    if d_emit and d_emit[0] <= (kc - 3) // 2:
                        pr = d_emit.pop(0)
                        d_mm(pr, pairs[pr])
                    if kc == 2 and finish_prev is not None:
                        finish_prev()
                for item in pending:
                    consume(*item)
                for pr in d_emit:
                    d_mm(pr, pairs[pr])

                while interleave:
                    out_proj(interleave.pop(0))

                def finish():
                    for qq in range(2):
                        # 1/d via exp(-ln(d)) on ACT: frees the PSUM
                        # accumulators fast and keeps DVE reciprocal (which
                        # measures ~6 cyc/elem) off the critical path.
                        lnd = p_sm.tile([128, 512], f32, tag="lnd", name="lnd")
                        nc.scalar.activation(out=lnd, in_=d_ps[qq], func=Ln)
                        rdb = p_sm.tile([128, 512], f32, tag="rdb", name="rdb")
                        nc.scalar.activation(out=rdb, in_=lnd, func=Exp, scale=-1.0)
                        nc.vector.tensor_mul(
                            oT_sb[h][:, q0 + qq * 512:q0 + (qq + 1) * 512],
                            o_ps[qq],
                            rdb,
                        )

                return finish

            fin = None
            for h in range(HPG):
                fin = attention_hh(h, 0, [], fin)
            sc_queue = list(range(8))
            for h in range(HPG):
                fin = attention_hh(h, 1, [sc_queue.pop(0), sc_queue.pop(0)], fin)
            fin()
            for sc in range(8, NSC):
                out_proj(sc)

    _split_waits(nc, mybir, maxw=1)
    return nc


def _prep_core_inputs(x, attn_mask, qkv_w, qkv_b, q_w, q_b, k_w, k_b, v_w, v_b,
                      out_w, use_dr=USE_DR):
    """Host-side: fold projections, shard, pre-transpose/tile, cast."""
    f = np.float32
    x = np.asarray(x, f)
    qkv_w = np.asarray(qkv_w, f)
    qkv_b = np.asarray(qkv_b, f)
    Ws = {}
    bs = {}
    for i, (w, b) in enumerate(((q_w, q_b), (k_w, k_b), (v_w, v_b))):
        w = np.asarray(w, f)
        b = np.asarray(b, f)
        sl = slice(i * D, (i + 1) * D)
        Ws[i] = w @ qkv_w[sl]              # [D, D] effective
        bs[i] = b + w @ qkv_b[sl]          # [D]
    out_wT = np.ascontiguousarray(np.asarray(out_w, f).T)  # [D(hd), D(model)]

    keepT = (np.asarray(attn_mask).T == 0).astype(BF16)    # [k, q]
    keepT_t = np.ascontiguousarray(keepT).reshape(NKC, 128, S)

    def dr_pack(mat_T, scale):
        # mat_T: [D, N] (contraction-major) -> [NDP, 128, 2, N] e4m3 * scale
        m = (mat_T * scale).reshape(NDP, 2, 128, -1).transpose(0, 2, 1, 3)
        return np.ascontiguousarray(m.astype(F8E4))

    xT_all = []
    x8_all = []
    for b_i in range(B):
        xb = np.ascontiguousarray(x[b_i].T)                # [D, S] f32
        xT_all.append(np.ascontiguousarray(xb.astype(BF16)).reshape(ND, 128, S))
        if use_dr:
            x8_all.append(dr_pack(xb, X_SCALE))

    maps = []
    for c in range(8):
        b_i = c % B
        g = c // B
        sl = slice(g * GD, (g + 1) * GD)
        m = {
            "xT": xT_all[b_i],
            "wvT": np.ascontiguousarray(Ws[2][sl].T.astype(BF16)).reshape(ND, 128, GD),
            "bq": np.ascontiguousarray(bs[0][sl].reshape(HPG, 128).T.astype(f)),
            "bk": np.ascontiguousarray(bs[1][sl].reshape(HPG, 128).T.astype(f)),
            "bvrow": bs[2][sl].astype(BF16).reshape(1, GD),
            "outwT": np.ascontiguousarray(out_wT[sl].astype(BF16)).reshape(HPG, 128, D),
            "keepT": keepT_t,
        }
        if use_dr:
            m["x8"] = x8_all[b_i]
            m["wq8"] = dr_pack(np.ascontiguousarray(Ws[0][sl].T), W_SCALE)
            m["wk8"] = dr_pack(np.ascontiguousarray(Ws[1][sl].T), W_SCALE)
        else:
            m["wqT"] = np.ascontiguousarray(Ws[0][sl].T.astype(BF16)).reshape(ND, 128, GD)
            m["wkT"] = np.ascontiguousarray(Ws[1][sl].T.astype(BF16)).reshape(ND, 128, GD)
        maps.append(m)
    return maps


def kernel(x, attn_mask, qkv_w, qkv_b, q_w, q_b, k_w, k_b, v_w, v_b,
           out_w, out_b, _trace=False):
    _install_ntff_hook_shim()
    from concourse.bass_utils import run_bass_kernel_spmd

    in_maps = _prep_core_inputs(
        x, attn_mask, qkv_w, qkv_b, q_w, q_b, k_w, k_b, v_w, v_b, out_w
    )
    use_vbias = bool(np.any(np.asarray(in_maps[0]["bvrow"], np.float32) != 0))
    key = ("nc", use_vbias)
    if key not in _cached:
        _cached[key] = _build_program(use_vbias=use_vbias)
    nc = _cached[key]
    core_ids = list(range(8))
    try:
        res = run_bass_kernel_spmd(nc, in_maps, core_ids, trace=_trace)
    except Exception:
        # transient NRT device wedge recovers on retry
        res = run_bass_kernel_spmd(nc, in_maps, core_ids, trace=_trace)
    _cached["last_result"] = res

    out_b = np.asarray(out_b, np.float32)
    full = np.empty((B, S, D), np.float32)
    for b_i in range(B):
        full[b_i] = (
            res.results[b_i]["out"].astype(np.float32)
            + res.results[b_i + B]["out"].astype(np.float32)
            + out_b
        )
    return full
